# revision 7
# baseline (speedup 1.0000x reference)
"""Trainium2 Bass kernel for nn_CachedAttention (8-core SPMD, tensor-parallel heads).

Contract: kernel(**inputs) takes the FULL unsharded inputs from
reference.setup_inputs() and returns the FULL (1, 2048, 2048) f32 output.

Math notes (validated against the reference in f32 at ~7e-6 rel err):
- The reference applies a TOP-LEFT-aligned causal mask tril(T, S) over the
  concatenated [cache; new] sequence, so new token t only attends to
  positions 0..t — all inside the 2048-entry cache. The freshly projected
  k/v (wk, wv, k-norm, k-rope) are therefore completely masked out and
  never computed here.
- RMSNorm's per-token scale commutes with RoPE (both linear), and q_norm_w
  folds into the RoPE cos/sin tables:
      out = q * C + swap_halves(q) * S'
      C[t,d]    = w[d] * cos(ang[t, d%64])
      S'[t,d<64]= -w[d+64] * sin(ang[t,d]);  S'[t,d>=64] = w[d-64] * sin(ang[t,d-64])
- Scores ~ N(0,1), so softmax runs without the max-subtraction pass; the
  row sum comes free from a ones-column appended to V.
- Sharding: core c owns q heads {2c, 2c+1} and kv head c. After attention,
  each core holds attnT per head (128 feat, 2048 t); one AllGather per head
  (overlapping the other head's attention) stacks these across cores, and
  core c computes output columns [256c, 256(c+1)) of the final wo
  projection against host-reordered wo rows. Host concatenates columns.

Perf notes:
- A tiny AllGather at kernel start absorbs the large one-time collective
  arming cost, overlapped with the q-projection.
- ScalarE runs only Square/Sqrt/Exp in batched runs (2-3 table loads
  total); exp skips the fully-masked below-diagonal region.
- wo runs in two passes: the head-0 half overlaps head-1's AllGather.
"""

import math
import sys

import numpy as np

sys.path.insert(0, "/opt/trn_rl_repo")

import ml_dtypes

P = 128
T = 2048
DM = 2048
DK = 128
HLOC = 2          # q heads per core
NCORES = 8
NT = T // P       # 16 token tiles
ND = DM // P      # 16 contraction chunks
NS = T // P       # 16 cache s-tiles
GW = 4            # token tiles per attention group (512 wide)
NG = NT // GW     # 4 groups
EPS = 1e-6
ROPE_BASE = 10000.0

_bf16 = ml_dtypes.bfloat16


def _build_module():
    import concourse.tile as tile
    from concourse import bacc, mybir

    bf = mybir.dt.bfloat16
    f32 = mybir.dt.float32
    AF = mybir.ActivationFunctionType

    nc = bacc.Bacc("TRN2", target_bir_lowering=False, debug=False, num_devices=NCORES)

    xT = nc.dram_tensor("xT", [DM, T], bf, kind="ExternalInput").ap()
    wqT = nc.dram_tensor("wqT", [DM, HLOC * DK], bf, kind="ExternalInput").ap()
    kcT = nc.dram_tensor("kcT", [DK, T], bf, kind="ExternalInput").ap()
    vca = nc.dram_tensor("vca", [T, DK + 1], bf, kind="ExternalInput").ap()
    woT = nc.dram_tensor("woT", [DM, HLOC * DK], bf, kind="ExternalInput").ap()
    cosw = nc.dram_tensor("cosw", [T, DK], bf, kind="ExternalInput").ap()
    sinw = nc.dram_tensor("sinw", [T, DK], bf, kind="ExternalInput").ap()
    tri = nc.dram_tensor("tri", [P, P], bf, kind="ExternalInput").ap()
    ident = nc.dram_tensor("ident", [P, P], bf, kind="ExternalInput").ap()
    out = nc.dram_tensor("out", [T, HLOC * DK], f32, kind="ExternalOutput").ap()

    with tile.TileContext(nc) as tc:
        with (
            tc.tile_pool(name="res", bufs=1) as res,
            tc.tile_pool(name="xpool", bufs=2) as xpool,
            tc.tile_pool(name="work", bufs=4) as work,
            tc.tile_pool(name="probs", bufs=18) as probs_pool,
            tc.tile_pool(name="small", bufs=6) as small,
            tc.tile_pool(name="outp", bufs=3) as outp,
            tc.tile_pool(name="ps_q", bufs=2, space="PSUM") as ps_q,
            tc.tile_pool(name="ps_tr", bufs=2, space="PSUM") as ps_tr,
            tc.tile_pool(name="ps_s", bufs=2, space="PSUM") as ps_s,
            tc.tile_pool(name="ps_o", bufs=2, space="PSUM") as ps_o,
            tc.tile_pool(name="dram", bufs=1, space="DRAM") as dram,
        ):
            # ---- phase-B-critical loads first (emission order ~ priority) ----
            wq_sb = res.tile([P, ND, HLOC * DK], bf)
            nc.sync.dma_start(wq_sb, wqT.rearrange("(o p) f -> p o f", p=P))
            cos_sb = res.tile([P, NT, DK], bf)
            nc.sync.dma_start(cos_sb, cosw.rearrange("(t p) d -> p t d", p=P))
            sin_sb = res.tile([P, NT, DK], bf)
            nc.sync.dma_start(sin_sb, sinw.rearrange("(t p) d -> p t d", p=P))
            id_sb = res.tile([P, P], bf)
            nc.sync.dma_start(id_sb, ident)
            eps_sb = res.tile([P, 1], f32)
            nc.vector.memset(eps_sb, EPS)

            # Warm up the collective path: the first collective in a NEFF
            # pays a large one-time arming cost; absorb it here, overlapped
            # with the q-projection phase.
            warm_in = dram.tile([P, 1], bf, name="warm_in")
            warm_out = dram.tile(
                [NCORES * P, 1], bf, addr_space="Shared", name="warm_out")
            warm_sb = res.tile([P, 1], bf)
            nc.vector.memset(warm_sb, 0.0)
            nc.sync.dma_start(warm_in, warm_sb)
            nc.gpsimd.collective_compute(
                "AllGather",
                mybir.AluOpType.bypass,
                ins=[warm_in.opt()],
                outs=[warm_out.opt()],
                replica_groups=[list(range(NCORES))],
            )

            qT = [res.tile([P, T], bf, name=f"qT{h}") for h in range(HLOC)]
            attnT = [res.tile([P, T], bf, name=f"attnT{h}") for h in range(HLOC)]
            qr_all = res.tile([P, NT * HLOC, DK], bf)
            ssq_all = res.tile([P, NT * HLOC], f32)
            rstd_all = res.tile([P, NT * HLOC], f32)

            # ---- phase B: q projection + rope (rstd deferred) ----
            TCH = 512
            xT_r = xT.rearrange("(o p) t -> p o t", p=P)
            for tci in range(T // TCH):
                x_sb = xpool.tile([P, ND, TCH], bf)
                nc.sync.dma_start(x_sb, xT_r[:, :, tci * TCH:(tci + 1) * TCH])
                for tj in range(TCH // P):
                    ti = tci * (TCH // P) + tj
                    pq = ps_q.tile([P, HLOC * DK], f32, tag="psq")
                    for dc in range(ND):
                        nc.tensor.matmul(
                            pq,
                            lhsT=x_sb[:, dc, tj * P:(tj + 1) * P],
                            rhs=wq_sb[:, dc, :],
                            start=(dc == 0),
                            stop=(dc == ND - 1),
                        )
                    qsb = work.tile([P, HLOC * DK], bf, tag="qsb")
                    nc.vector.tensor_copy(qsb, pq)
                    for h in range(HLOC):
                        idx = ti * HLOC + h
                        qh = qsb[:, h * DK:(h + 1) * DK]
                        # sumsq on ScalarE (idle in this phase); scratch unused
                        qsq = work.tile([P, DK], bf, tag="qsq")
                        nc.scalar.activation(
                            out=qsq, in_=pq[:, h * DK:(h + 1) * DK],
                            func=AF.Square,
                            accum_out=ssq_all[:, idx:idx + 1])
                        # rope: qr = q*C + swap_halves(q)*S'   (bf16 on DVE)
                        qr = qr_all[:, idx, :]
                        u = work.tile([P, DK], bf, tag="u")
                        nc.vector.tensor_mul(
                            u[:, :DK // 2], qh[:, DK // 2:], sin_sb[:, ti, :DK // 2])
                        nc.vector.tensor_mul(
                            u[:, DK // 2:], qh[:, :DK // 2], sin_sb[:, ti, DK // 2:])
                        t1 = work.tile([P, DK], bf, tag="t1")
                        nc.vector.tensor_mul(t1, qh, cos_sb[:, ti, :])
                        nc.vector.tensor_add(qr, t1, u)

            # batched rstd: one Sqrt + one reciprocal for all 32 (ti, h)
            nc.scalar.activation(
                out=ssq_all, in_=ssq_all, func=AF.Sqrt,
                bias=eps_sb, scale=1.0 / DK)
            nc.vector.reciprocal(rstd_all, ssq_all)

            for ti in range(NT):
                for h in range(HLOC):
                    idx = ti * HLOC + h
                    qrs = work.tile([P, DK], bf, tag="qrs")
                    nc.vector.tensor_scalar_mul(
                        qrs, qr_all[:, idx, :], rstd_all[:, idx:idx + 1])
                    ptr = ps_tr.tile([P, P], bf, tag="ptr")
                    nc.tensor.transpose(ptr, qrs, id_sb)
                    nc.vector.tensor_copy(qT[h][:, ti * P:(ti + 1) * P], ptr)

            # ---- attention-phase loads ----
            kc_sb = res.tile([P, T], bf)
            nc.sync.dma_start(kc_sb, kcT)
            vca_sb = res.tile([P, NS, DK + 1], bf)
            nc.sync.dma_start(vca_sb, vca.rearrange("(s p) d -> p s d", p=P))
            tri_sb = res.tile([P, P], bf)
            nc.sync.dma_start(tri_sb, tri)
            wo_sb = res.tile([P, ND, HLOC * DK], bf)
            nc.sync.dma_start(wo_sb, woT.rearrange("(o p) f -> p o f", p=P))

            # ---- phase C: attention (512-wide score/exp groups) ----
            for h in range(HLOC):
                for g in range(NG):
                    t0 = g * GW * P
                    pb_tiles = []
                    for si in range(GW * (g + 1)):
                        k = max(0, si - g * GW)  # skip below-diagonal tiles
                        ps = ps_s.tile([P, GW * P], f32, tag="ps")
                        nc.tensor.matmul(
                            ps[:, k * P:],
                            lhsT=kc_sb[:, si * P:(si + 1) * P],
                            rhs=qT[h][:, t0 + k * P:t0 + GW * P],
                            start=True, stop=True,
                        )
                        pb = probs_pool.tile([P, GW * P], bf, tag="pb")
                        nc.scalar.activation(
                            out=pb[:, k * P:], in_=ps[:, k * P:], func=AF.Exp)
                        if si >= g * GW:
                            nc.vector.tensor_mul(
                                pb[:, k * P:(k + 1) * P],
                                pb[:, k * P:(k + 1) * P], tri_sb)
                        pb_tiles.append(pb)
                    for tj in range(GW):
                        ti = g * GW + tj
                        po = ps_o.tile([P, DK + 1], f32, tag="po")
                        for si in range(ti + 1):
                            nc.tensor.matmul(
                                po,
                                lhsT=pb_tiles[si][:, tj * P:(tj + 1) * P],
                                rhs=vca_sb[:, si, :],
                                start=(si == 0), stop=(si == ti),
                            )
                        recip = small.tile([P, 1], f32, tag="recip")
                        nc.vector.reciprocal(recip, po[:, DK:DK + 1])
                        at = work.tile([P, DK], bf, tag="at")
                        nc.vector.tensor_scalar_mul(at, po[:, :DK], recip)
                        ptr2 = ps_tr.tile([P, P], bf, tag="ptr")
                        nc.tensor.transpose(ptr2, at, id_sb)
                        nc.vector.tensor_copy(
                            attnT[h][:, ti * P:(ti + 1) * P], ptr2)

            # ---- phase D: per-head AllGather (head 0 overlaps head 1 attn) ----
            af_sb = []
            for h in range(HLOC):
                ag_in = dram.tile([P, T], bf, name=f"ag_in{h}")
                ag_out = dram.tile(
                    [NCORES * P, T], bf, addr_space="Shared", name=f"ag_out{h}")
                nc.sync.dma_start(ag_in, attnT[h])
                nc.gpsimd.collective_compute(
                    "AllGather",
                    mybir.AluOpType.bypass,
                    ins=[ag_in.opt()],
                    outs=[ag_out.opt()],
                    replica_groups=[list(range(NCORES))],
                )
                af = res.tile([P, NCORES, T], bf, name=f"af{h}")
                nc.sync.dma_start(af, ag_out.rearrange("(o p) t -> p o t", p=P))
                af_sb.append(af)

            # ---- phase E: wo projection, two passes so the head-0 half
            # overlaps head-1's AllGather ----
            wo0_sb = res.tile([P, NT, HLOC * DK], f32)
            for ti in range(NT):
                pout = ps_q.tile([P, HLOC * DK], f32, tag="psq")
                for fc in range(ND // 2):
                    nc.tensor.matmul(
                        pout,
                        lhsT=af_sb[0][:, fc, ti * P:(ti + 1) * P],
                        rhs=wo_sb[:, fc, :],
                        start=(fc == 0),
                        stop=(fc == ND // 2 - 1),
                    )
                nc.vector.tensor_copy(wo0_sb[:, ti, :], pout)
            out_r = out.rearrange("(t p) f -> p t f", p=P)
            for ti in range(NT):
                pout = ps_q.tile([P, HLOC * DK], f32, tag="psq")
                for fc in range(ND // 2, ND):
                    nc.tensor.matmul(
                        pout,
                        lhsT=af_sb[1][:, fc - ND // 2, ti * P:(ti + 1) * P],
                        rhs=wo_sb[:, fc, :],
                        start=(fc == ND // 2),
                        stop=(fc == ND - 1),
                    )
                osb = outp.tile([P, HLOC * DK], f32, tag="osb")
                nc.vector.tensor_add(osb, pout, wo0_sb[:, ti, :])
                nc.sync.dma_start(out_r[:, ti, :], osb)

    nc.compile()
    return nc


def _host_inputs(x, cached_k, cached_v, wq, wo, q_norm_w):
    """Build the 8 per-core input maps (host-side shard + fold + cast)."""
    xt = np.ascontiguousarray(x[0].T).astype(_bf16)           # (DM, T)

    inv_freq = 1.0 / (ROPE_BASE ** (np.arange(0, DK, 2, dtype=np.float32) / DK))
    ang = np.arange(T, dtype=np.float32)[:, None] * inv_freq[None, :]
    cos_f = np.concatenate([np.cos(ang), np.cos(ang)], axis=1)
    sin_f = np.concatenate([np.sin(ang), np.sin(ang)], axis=1)
    w = q_norm_w.astype(np.float32)
    C = (w[None, :] * cos_f).astype(_bf16)
    Sp = np.empty((T, DK), np.float32)
    Sp[:, :DK // 2] = -w[None, DK // 2:] * sin_f[:, :DK // 2]
    Sp[:, DK // 2:] = w[None, :DK // 2] * sin_f[:, DK // 2:]
    Sp = Sp.astype(_bf16)

    tri = (np.arange(P)[:, None] <= np.arange(P)[None, :]).astype(_bf16)
    ident = np.eye(P, dtype=_bf16)

    # wo row order must match the two per-head gathers:
    # phase E contraction index fc*128+d with fc = h*8 + o reads gathered
    # feature (2*o + h)*128 + d of the original attn layout.
    perm = np.empty(DM, np.int64)
    for h in range(HLOC):
        for o in range(NCORES):
            src = (HLOC * o + h) * DK
            dst = (h * NCORES + o) * DK
            perm[dst:dst + DK] = np.arange(src, src + DK)

    in_maps = []
    for c in range(NCORES):
        fs = slice(c * HLOC * DK, (c + 1) * HLOC * DK)
        wqT = np.ascontiguousarray(wq[fs, :].T).astype(_bf16)
        woT = np.ascontiguousarray(wo[fs, :].T[perm]).astype(_bf16)
        kcT = np.ascontiguousarray(cached_k[c].T / math.sqrt(DK)).astype(_bf16)
        vcaa = np.concatenate(
            [cached_v[c], np.ones((T, 1), np.float32)], axis=1).astype(_bf16)
        in_maps.append({
            "xT": xt, "wqT": wqT, "kcT": kcT, "vca": vcaa, "woT": woT,
            "cosw": C, "sinw": Sp, "tri": tri, "ident": ident,
        })
    return in_maps


_CACHED = {}


def _get_module():
    if "nc" not in _CACHED:
        _CACHED["nc"] = _build_module()
    return _CACHED["nc"]


def run(inputs, trace=False, **kw):
    """Compile (cached), run on 8 cores, return (output, BassKernelResults)."""
    from concourse import bass_utils

    nc = _get_module()
    in_maps = _host_inputs(
        np.asarray(inputs["x"], np.float32),
        np.asarray(inputs["cached_k"], np.float32),
        np.asarray(inputs["cached_v"], np.float32),
        np.asarray(inputs["wq"], np.float32),
        np.asarray(inputs["wo"], np.float32),
        np.asarray(inputs["q_norm_w"], np.float32),
    )
    res = bass_utils.run_bass_kernel_spmd(
        nc, in_maps, core_ids=list(range(NCORES)), trace=trace, **kw)
    cols = [res.results[c]["out"] for c in range(NCORES)]
    full = np.concatenate(cols, axis=1).reshape(1, T, DM).astype(np.float32)
    return full, res


def kernel(**inputs):
    full, _ = run(inputs)
    return full
